# revision 43
# baseline (speedup 1.0000x reference)
"""RNN-T joint network (dense MLP) Trainium2 Bass kernel.

Math (per batch row n):
    h = relu(x @ W1.T + b1)     x = concat(f, g)   # [N, 512]
    y = h @ W2.T + b2                              # [N, 29]

Strategy: data-parallel over batch N=32768 across 8 NeuronCores (4096
rows/core); weights replicated.

Both layers run on the PE in fp8e4 (e4m3) DoubleRow mode (0.5
cycles/row, 2 k-rows per slot-pair) with 3-term error compensation:

    x ~= x8 + xlo          (both e4m3, exact split of the f32 value)
    W ~= (Whi + Wlo)/S     (e4m3 at device scale S; Wlo = residual)
    x @ W = (x8 @ (Whi + Wlo) + xlo @ Whi) / S     [+ O(eps^2) dropped]

Layer 1: every DoubleRow matmul carries two k-tiles (one per slot), so
per j-tile and 512-row chunk layer 1 is 12 DR instructions covering
K=1344 (10 full 128-tiles + one 64-row tile packed by stacking
[Whi;Wlo] vertically in one slot), at 256 PE cycles each — 3072 cycles
vs 5632 for the fp32r baseline: a-terms (x8 @ Whi, 5 DR), the b-term
(x8 @ Wlo) only for the g-part k-tile pair 8-9 whose weights have 3.2x
the variance of the f-part (dropping the f-part Wlo raises the measured
rel err to a still-safe 1.52e-2), the stacked k-tile-10 DR, and c-terms
(xlo @ Whi, 5 DR) which reuse the a-term weight tiles in SBUF.

Layer 2 (chunks 0..6): h is split on-device into h8 (2nd ScalarE
activation pass, fp8 out) + hlo (VectorE subtract, fp8 out) and the
29-wide projection runs as 6 DR matmuls (1536 cycles vs 2048 fp32r).
The last chunk keeps the fp32r path so the drain doesn't wait on the
h8/hlo chain.  Labels padded 29->32 (DR weight AP step must be %16).

Timeline tuning (cost-model driven): the PE p-state ramp resets on any
engine gap (3us of half-clock matmuls each time), so scratch "warmup"
DR matmuls bridge the DMA fill and the final-chunk activation latency;
the fill DMAs ride one queue (SP) in just-in-time order.
"""

import numpy as np
import ml_dtypes

import concourse.bacc as bacc
import concourse.bass as bass  # noqa: F401
import concourse.mybir as mybir
from concourse import tile
from concourse.bass_utils import run_bass_kernel_spmd

TRANS_H, PRED_H, JOINT_H, NUM_LABELS = 1024, 320, 512, 29
LAB_PAD = 32
BATCH = 32768
N_CORES = 8
N_PER_CORE = BATCH // N_CORES          # 4096
K_TOTAL = TRANS_H + PRED_H             # 1344 = 10*128 + 64
K_FULL = 10                            # full 128-row k-tiles
K_REM = K_TOTAL - K_FULL * 128         # 64
J_TILES = JOINT_H // 128               # 4
N_CHUNK = 512                          # PSUM-bank / fp32r moving limit
N_CHUNKS = N_PER_CORE // N_CHUNK       # 8
X_SLOTS = 22                           # 10 x8 + x8dup + xlohalf + 10 xlo
N_DR = 11                              # weight DR slots per j (a:5, b:5, t10:1)
W_SCALE = 4096.0
W2_SCALE = 2048.0
N_WARM = 14                            # fill-bridging PE warmup matmuls
N_WARM_TAIL = 4                        # drain-bridging warmups (chunk 7 l2)

F32 = mybir.dt.float32
F32R = mybir.dt.float32r
F8 = mybir.dt.float8e4
DR = mybir.MatmulPerfMode.DoubleRow
E4NP = ml_dtypes.float8_e4m3

_NC_CACHE = {}


def _build_bass():
    """Single-core Bass program (same NEFF runs SPMD on 8 cores)."""
    nc = bacc.Bacc(None)

    xq = nc.dram_tensor("xq", [X_SLOTS * 128, N_PER_CORE], F8, kind="ExternalInput")
    w1 = nc.dram_tensor("w1", [128, J_TILES, N_DR, 2, 128], F8, kind="ExternalInput")
    b1 = nc.dram_tensor("b1", [JOINT_H, 1], F32, kind="ExternalInput")
    w2T = nc.dram_tensor("w2T", [JOINT_H, NUM_LABELS], F32R, kind="ExternalInput")
    w2dr = nc.dram_tensor("w2dr", [128, 4, 2, LAB_PAD], F8, kind="ExternalInput")
    b2 = nc.dram_tensor("b2", [NUM_LABELS, 1], F32, kind="ExternalInput")
    yT = nc.dram_tensor("yT", [NUM_LABELS, N_PER_CORE], F32, kind="ExternalOutput")

    xq3 = xq.rearrange("(s p) n -> p s n", p=128)     # [128, 22, N]

    with tile.TileContext(nc) as tc:
        with (
            tc.tile_pool(name="consts", bufs=1) as consts,
            tc.tile_pool(name="xpool", bufs=3) as xpool,
            tc.tile_pool(name="lopool", bufs=3) as lopool,
            tc.tile_pool(name="hpool", bufs=2) as hpool,
            tc.tile_pool(name="h8pool", bufs=2) as h8pool,
            tc.tile_pool(name="opool", bufs=2) as opool,
            tc.tile_pool(name="psum_h", bufs=5, space="PSUM") as psum_h,
            tc.tile_pool(name="psum_y", bufs=2, space="PSUM") as psum_y,
            tc.tile_pool(name="psum_w", bufs=1, space="PSUM") as psum_w,
        ):
            # ---- constants: single SP queue, just-in-time order ----
            w1_sb = consts.tile([128, J_TILES, N_DR, 2, 128], F8, name="w1_sb", tag="w1")
            b1_sb = consts.tile([128, J_TILES], F32, name="b1_sb", tag="b1")
            w2_sb = consts.tile([128, J_TILES, NUM_LABELS], F32R, name="w2_sb", tag="w2")
            w2dr_sb = consts.tile([128, 4, 2, LAB_PAD], F8, name="w2dr_sb", tag="w2dr")
            b2_sb = consts.tile([NUM_LABELS, 1], F32, name="b2_sb", tag="b2")

            x8_c0 = xpool.tile([128, 12, N_CHUNK], F8, name="x8_sb", tag="x8")
            xlo_c0 = lopool.tile([128, 10, N_CHUNK], F8, name="xlo_sb", tag="xlo")

            nc.sync.dma_start(out=w1_sb[:, 0], in_=w1[:, 0])
            nc.sync.dma_start(out=x8_c0[:, 0:10], in_=xq3[:, 0:10, 0:N_CHUNK])
            nc.sync.dma_start(out=x8_c0[:, 10:12], in_=xq3[:, 10:12, 0:N_CHUNK])
            nc.sync.dma_start(out=xlo_c0[:, 0:6], in_=xq3[:, 12:18, 0:N_CHUNK])
            nc.sync.dma_start(out=b1_sb, in_=b1.rearrange("(j p) o -> p (j o)", p=128))
            nc.sync.dma_start(out=w1_sb[:, 1], in_=w1[:, 1])
            nc.sync.dma_start(out=xlo_c0[:, 6:10], in_=xq3[:, 18:22, 0:N_CHUNK])
            nc.sync.dma_start(out=w1_sb[:, 2], in_=w1[:, 2])
            nc.sync.dma_start(out=w1_sb[:, 3], in_=w1[:, 3])
            xlo_c1 = lopool.tile([128, 10, N_CHUNK], F8, name="xlo_sb", tag="xlo")
            nc.sync.dma_start(out=xlo_c1[:, 0:6], in_=xq3[:, 12:18, N_CHUNK:2 * N_CHUNK])
            nc.sync.dma_start(out=xlo_c1[:, 6:10], in_=xq3[:, 18:22, N_CHUNK:2 * N_CHUNK])
            x8_c1 = xpool.tile([128, 12, N_CHUNK], F8, name="x8_sb", tag="x8")
            nc.sync.dma_start(out=x8_c1, in_=xq3[:, 0:12, N_CHUNK:2 * N_CHUNK])
            nc.sync.dma_start(out=w2dr_sb, in_=w2dr[:, :, :, :])
            nc.sync.dma_start(out=b2_sb, in_=b2[:, :])
            nc.sync.dma_start(
                out=w2_sb, in_=w2T.rearrange("(j p) l -> p j l", p=128)
            )

            # ---- PE warmup: scratch DR matmuls keep the p-state ramp hot
            # across the DMA fill (any PE gap resets 3us of full clock).
            warm_sb = consts.tile([128, 2, N_CHUNK], F8, name="warm_sb", tag="warm")
            nc.vector.memset(warm_sb, 0.0)
            pw = psum_w.tile([128, N_CHUNK], F32, name="pw", tag="pw")

            def emit_warmups(n):
                for _ in range(n):
                    nc.tensor.matmul(
                        pw, lhsT=warm_sb[:, :, 0:128], rhs=warm_sb,
                        start=True, stop=True, perf_mode=DR,
                    )

            emit_warmups(N_WARM)

            # ---- main loop over batch chunks of 512 rows ----
            # Layer 2 of chunk c-1 is emitted mid-chunk c so the PE never
            # idles on the j3 activation / h8 / hlo chain.
            pending = None  # (h_tiles, h8_pairs, hlo_pairs, n0) of prev chunk
            for c in range(N_CHUNKS):
                n0 = c * N_CHUNK
                if c > 1:
                    x8_sb = xpool.tile([128, 12, N_CHUNK], F8, name="x8_sb", tag="x8")
                    nc.sync.dma_start(out=x8_sb, in_=xq3[:, 0:12, n0:n0 + N_CHUNK])
                    xlo_sb = lopool.tile([128, 10, N_CHUNK], F8, name="xlo_sb", tag="xlo")
                    nc.sync.dma_start(out=xlo_sb[:, 0:6], in_=xq3[:, 12:18, n0:n0 + N_CHUNK])
                    nc.sync.dma_start(out=xlo_sb[:, 6:10], in_=xq3[:, 18:22, n0:n0 + N_CHUNK])
                elif c == 1:
                    x8_sb, xlo_sb = x8_c1, xlo_c1
                else:
                    x8_sb, xlo_sb = x8_c0, xlo_c0

                last = c == N_CHUNKS - 1
                h_tiles, h8_pairs, hlo_pairs = [], [], []
                if not last:
                    for jp in range(2):
                        h8_pairs.append(h8pool.tile(
                            [128, 2, N_CHUNK], F8, name=f"h8_{jp}", tag=f"h8_{jp}"))
                        hlo_pairs.append(h8pool.tile(
                            [128, 2, N_CHUNK], F8, name=f"hlo_{jp}", tag=f"hlo_{jp}"))
                def emit_block(j, ph_out, c0, c1):
                    # a-terms: x8 @ Whi, 5 DR pairs over k-tiles 0..9
                    for q in range(5):
                        nc.tensor.matmul(
                            ph_out, lhsT=w1_sb[:, j, q],
                            rhs=x8_sb[:, 2 * q:2 * q + 2, c0:c1],
                            start=(q == 0), stop=False, perf_mode=DR,
                        )
                    # b-term: x8 @ Wlo for the g-part k-tile pair (8,9)
                    # only — f-part weights have 3.2x less variance and their
                    # Wlo is dropped (measured rel err 1.52e-2 vs 2e-2 gate)
                    nc.tensor.matmul(
                        ph_out, lhsT=w1_sb[:, j, 5], rhs=x8_sb[:, 8:10, c0:c1],
                        start=False, stop=False, perf_mode=DR,
                    )
                    # k-tile 10 (64 rows): slot0 [Whi10;Wlo10]@[x8;x8],
                    # slot1 [Whi10;0]@[xlo;0]
                    nc.tensor.matmul(
                        ph_out, lhsT=w1_sb[:, j, 6], rhs=x8_sb[:, 10:12, c0:c1],
                        start=False, stop=False, perf_mode=DR,
                    )
                    # c-terms: xlo @ Whi (reuses a-term weight tiles)
                    for q in range(5):
                        nc.tensor.matmul(
                            ph_out, lhsT=w1_sb[:, j, q],
                            rhs=xlo_sb[:, 2 * q:2 * q + 2, c0:c1],
                            start=False, stop=(q == 4), perf_mode=DR,
                        )

                c1_phs = None
                if c == 1:
                    # c-first: run xlo-only terms while x8_c1 is in flight
                    c1_phs = [psum_h.tile([128, N_CHUNK], F32,
                                          name=f"ph_{j}", tag="ph")
                              for j in range(J_TILES)]
                    for j in range(J_TILES):
                        for q in range(5):
                            nc.tensor.matmul(
                                c1_phs[j], lhsT=w1_sb[:, j, q],
                                rhs=xlo_sb[:, 2 * q:2 * q + 2, :],
                                start=(q == 0), stop=False, perf_mode=DR,
                            )
                for j in range(J_TILES):
                    if c == 1:
                        ph = c1_phs[j]
                        for q in range(5):
                            nc.tensor.matmul(
                                ph, lhsT=w1_sb[:, j, q],
                                rhs=x8_sb[:, 2 * q:2 * q + 2, :],
                                start=False, stop=False, perf_mode=DR,
                            )
                        nc.tensor.matmul(
                            ph, lhsT=w1_sb[:, j, 5], rhs=x8_sb[:, 8:10, :],
                            start=False, stop=False, perf_mode=DR,
                        )
                        nc.tensor.matmul(
                            ph, lhsT=w1_sb[:, j, 6], rhs=x8_sb[:, 10:12, :],
                            start=False, stop=True, perf_mode=DR,
                        )
                        h_sb = hpool.tile([128, N_CHUNK], F32R,
                                          name=f"h_{j}", tag=f"h_{j}")
                        nc.scalar.activation(
                            h_sb, ph, mybir.ActivationFunctionType.Relu,
                            bias=b1_sb[:, j:j + 1], scale=1.0 / W_SCALE,
                        )
                        h_tiles.append(h_sb)
                        h8 = h8_pairs[j // 2][:, j % 2, :]
                        nc.scalar.activation(
                            h8, ph, mybir.ActivationFunctionType.Relu,
                            bias=b1_sb[:, j:j + 1], scale=1.0 / W_SCALE,
                        )
                        nc.vector.tensor_tensor(
                            out=hlo_pairs[j // 2][:, j % 2, :], in0=h_sb, in1=h8,
                            op=mybir.AluOpType.subtract,
                        )
                        if j == 2 and pending is not None:
                            pending_py = _emit_l2_matmuls(nc, psum_y, w2dr_sb, *pending)
                        if j == 3 and pending is not None:
                            _emit_l2_out(nc, opool, b2_sb, yT, pending_py, pending[-1])
                            pending = None
                        continue
                    if last and j == 3:
                        # half-width groups on two banks: the 385ns half
                        # activations overlap the DR stream instead of
                        # gating the whole drain at 612ns
                        h_sb = hpool.tile([128, N_CHUNK], F32R,
                                          name="h_3", tag="h_3")
                        pha = psum_h.tile([128, N_CHUNK], F32, name="ph_3a", tag="ph")
                        emit_block(3, pha[:, 0:256], 0, 256)
                        nc.scalar.activation(
                            h_sb[:, 0:256], pha[:, 0:256],
                            mybir.ActivationFunctionType.Relu,
                            bias=b1_sb[:, 3:4], scale=1.0 / W_SCALE,
                        )
                        phb = psum_h.tile([128, N_CHUNK], F32, name="ph_3b", tag="ph")
                        emit_block(3, phb[:, 0:256], 256, 512)
                        nc.scalar.activation(
                            h_sb[:, 256:512], phb[:, 0:256],
                            mybir.ActivationFunctionType.Relu,
                            bias=b1_sb[:, 3:4], scale=1.0 / W_SCALE,
                        )
                        h_tiles.append(h_sb)
                        if pending is not None:
                            # c6's y bias+scale on DVE so it never contends
                            # with the final ACT chain
                            py6, n6 = pending_py, pending[-1]
                            y6 = opool.tile([NUM_LABELS, N_CHUNK], F32,
                                            name="y_sb", tag="y")
                            nc.vector.tensor_scalar(
                                out=y6, in0=py6[0:NUM_LABELS, :],
                                scalar1=1.0 / W2_SCALE, scalar2=b2_sb,
                                op0=mybir.AluOpType.mult,
                                op1=mybir.AluOpType.add,
                            )
                            nc.sync.dma_start(
                                out=yT[:, n6:n6 + N_CHUNK], in_=y6)
                            pending = None
                        continue
                    ph = psum_h.tile([128, N_CHUNK], F32, name=f"ph_{j}", tag="ph")
                    emit_block(j, ph, 0, N_CHUNK)
                    h_sb = hpool.tile([128, N_CHUNK], F32R, name=f"h_{j}", tag=f"h_{j}")
                    nc.scalar.activation(
                        h_sb, ph, mybir.ActivationFunctionType.Relu,
                        bias=b1_sb[:, j:j + 1], scale=1.0 / W_SCALE,
                    )
                    h_tiles.append(h_sb)
                    if not last:
                        h8 = h8_pairs[j // 2][:, j % 2, :]
                        nc.scalar.activation(
                            h8, ph, mybir.ActivationFunctionType.Relu,
                            bias=b1_sb[:, j:j + 1], scale=1.0 / W_SCALE,
                        )
                        nc.vector.tensor_tensor(
                            out=hlo_pairs[j // 2][:, j % 2, :], in0=h_sb, in1=h8,
                            op=mybir.AluOpType.subtract,
                        )
                    if j == 2 and pending is not None:
                        pending_py = _emit_l2_matmuls(nc, psum_y, w2dr_sb, *pending)
                    if j == 3 and pending is not None:
                        # y-act after act1-j3 so it never delays the h chain
                        _emit_l2_out(nc, opool, b2_sb, yT, pending_py, pending[-1])
                        pending = None

                pending = (h_tiles, h8_pairs, hlo_pairs, n0)

            # drain: bridge the final activation latency, then fp32r layer 2
            # for the last chunk (no h8/hlo dependency in the tail).
            emit_warmups(N_WARM_TAIL)
            h_tiles, _, _, n0 = pending
            y_sb = opool.tile([NUM_LABELS, N_CHUNK], F32, name="y_sb", tag="y")
            py_a = psum_y.tile([NUM_LABELS, 256], F32, name="py_a", tag="py")
            for j in range(J_TILES):
                nc.tensor.matmul(
                    py_a, lhsT=w2_sb[:, j, :], rhs=h_tiles[j][:, 0:256],
                    start=(j == 0), stop=(j == J_TILES - 1),
                )
            nc.scalar.activation(
                y_sb[:, 0:256], py_a, mybir.ActivationFunctionType.Identity,
                bias=b2_sb,
            )
            py_b = psum_y.tile([NUM_LABELS, 256], F32, name="py_b", tag="py")
            for j in range(J_TILES):
                nc.tensor.matmul(
                    py_b, lhsT=w2_sb[:, j, :], rhs=h_tiles[j][:, 256:512],
                    start=(j == 0), stop=(j == J_TILES - 1),
                )
            nc.vector.tensor_scalar_add(
                out=y_sb[:, 256:512], in0=py_b, scalar1=b2_sb
            )
            # final y rides SP (dge 650 vs 784 on ACT; SP is idle at drain)
            nc.sync.dma_start(out=yT[:, n0:n0 + N_CHUNK], in_=y_sb)

    nc.finalize()
    return nc


def _emit_l2_matmuls(nc, psum_y, w2dr_sb, h_tiles, h8_pairs, hlo_pairs, n0):
    """fp8 DR layer 2 matmuls: py = h8 @ (W2hi+W2lo) + hlo @ W2hi."""
    py = psum_y.tile([LAB_PAD, N_CHUNK], F32, name="py", tag="py")
    nc.tensor.matmul(py, lhsT=w2dr_sb[:, 0], rhs=h8_pairs[0],
                     start=True, stop=False, perf_mode=DR)
    nc.tensor.matmul(py, lhsT=w2dr_sb[:, 1], rhs=h8_pairs[1],
                     start=False, stop=False, perf_mode=DR)
    nc.tensor.matmul(py, lhsT=w2dr_sb[:, 2], rhs=h8_pairs[0],
                     start=False, stop=False, perf_mode=DR)
    nc.tensor.matmul(py, lhsT=w2dr_sb[:, 3], rhs=h8_pairs[1],
                     start=False, stop=False, perf_mode=DR)
    nc.tensor.matmul(py, lhsT=w2dr_sb[:, 0], rhs=hlo_pairs[0],
                     start=False, stop=False, perf_mode=DR)
    nc.tensor.matmul(py, lhsT=w2dr_sb[:, 1], rhs=hlo_pairs[1],
                     start=False, stop=True, perf_mode=DR)
    return py


def _emit_l2_out(nc, opool, b2_sb, yT, py, n0):
    y_sb = opool.tile([NUM_LABELS, N_CHUNK], F32, name="y_sb", tag="y")
    nc.scalar.activation(
        y_sb, py[0:NUM_LABELS, :], mybir.ActivationFunctionType.Identity,
        bias=b2_sb, scale=1.0 / W2_SCALE,
    )
    nc.scalar.dma_start(out=yT[:, n0:n0 + N_CHUNK], in_=y_sb)


def _get_nc():
    if "nc" not in _NC_CACHE:
        _NC_CACHE["nc"] = _build_bass()
    return _NC_CACHE["nc"]


def _q8(a):
    return np.asarray(a, dtype=E4NP)


def _prep_in_maps(f, g, W1t, b1t, W1p, b1p, W2, b2):
    f2 = np.asarray(f, np.float32).reshape(BATCH, TRANS_H)
    g2 = np.asarray(g, np.float32).reshape(BATCH, PRED_H)
    x = np.concatenate([f2, g2], axis=1)            # [BATCH, 1344]

    x8 = _q8(x)                                     # e4m3, device scale 1
    xlo = _q8(x - x8.astype(np.float32))            # e4m3 residual, scale 1

    W1 = np.concatenate(
        [np.asarray(W1t, np.float32), np.asarray(W1p, np.float32)], axis=1
    ).T                                             # [1344, 512]
    Whi = _q8(W1 * W_SCALE)                         # device scale 4096
    Wlo = _q8(W1 * W_SCALE - Whi.astype(np.float32))

    # layer-1 weight DR-pair tensor [p, j, dr, slot, col]
    w1dr = np.zeros((128, J_TILES, N_DR, 2, 128), dtype=E4NP)
    Whi_p = np.zeros((11 * 128, JOINT_H), dtype=E4NP)
    Wlo_p = np.zeros((11 * 128, JOINT_H), dtype=E4NP)
    Whi_p[:K_TOTAL] = Whi
    Wlo_p[:K_TOTAL] = Wlo
    for j in range(J_TILES):
        cols = slice(j * 128, (j + 1) * 128)
        for q in range(5):
            w1dr[:, j, q, 0] = Whi_p[(2 * q) * 128:(2 * q + 1) * 128, cols]
            w1dr[:, j, q, 1] = Whi_p[(2 * q + 1) * 128:(2 * q + 2) * 128, cols]
        w1dr[:, j, 5, 0] = Wlo_p[8 * 128:9 * 128, cols]
        w1dr[:, j, 5, 1] = Wlo_p[9 * 128:10 * 128, cols]
        # k-tile 10: slot0 = [Whi10; Wlo10] stacked, slot1 = [Whi10; 0]
        w1dr[:K_REM, j, 6, 0] = Whi[K_FULL * 128:, cols]
        w1dr[K_REM:2 * K_REM, j, 6, 0] = Wlo[K_FULL * 128:, cols]
        w1dr[:K_REM, j, 6, 1] = Whi[K_FULL * 128:, cols]

    # layer-2 weight DR tensor [p, {hi01,hi23,lo01,lo23}, slot, lab]
    W2T = np.asarray(W2, np.float32).T              # [512, 29]
    W2hi = _q8(W2T * W2_SCALE)
    W2lo = _q8(W2T * W2_SCALE - W2hi.astype(np.float32))
    w2drh = np.zeros((128, 4, 2, LAB_PAD), dtype=E4NP)
    for jp in range(2):
        for s in range(2):
            rows = slice((2 * jp + s) * 128, (2 * jp + s + 1) * 128)
            w2drh[:, jp, s, :NUM_LABELS] = W2hi[rows]
            w2drh[:, 2 + jp, s, :NUM_LABELS] = W2lo[rows]

    b1c = (np.asarray(b1t, np.float32) + np.asarray(b1p, np.float32)).reshape(
        JOINT_H, 1
    )
    w2c = np.ascontiguousarray(W2T)
    b2c = np.asarray(b2, np.float32).reshape(NUM_LABELS, 1)

    in_maps = []
    for core in range(N_CORES):
        sl = slice(core * N_PER_CORE, (core + 1) * N_PER_CORE)
        x8c = x8[sl]                                # [4096, 1344]
        xloc = xlo[sl]
        xqc = np.zeros((X_SLOTS, 128, N_PER_CORE), dtype=E4NP)
        for t in range(K_FULL):
            xqc[t] = x8c[:, t * 128:(t + 1) * 128].T
        # slot 10: x8 tile-10 duplicated vertically; slot 11: xlo tile-10 + 0s
        xqc[10, :K_REM] = x8c[:, K_FULL * 128:].T
        xqc[10, K_REM:] = x8c[:, K_FULL * 128:].T
        xqc[11, :K_REM] = xloc[:, K_FULL * 128:].T
        for t in range(K_FULL):
            xqc[12 + t] = xloc[:, t * 128:(t + 1) * 128].T
        in_maps.append({
            "xq": xqc.reshape(X_SLOTS * 128, N_PER_CORE),
            "w1": w1dr, "b1": b1c, "w2T": w2c, "w2dr": w2drh, "b2": b2c,
        })
    return in_maps


def _gather(results):
    y = np.empty((1, BATCH, NUM_LABELS), np.float32)
    for core, r in enumerate(results):
        y[0, core * N_PER_CORE:(core + 1) * N_PER_CORE] = r["yT"].T
    return y


def _run(inputs, trace=False):
    in_maps = _prep_in_maps(
        inputs["f"], inputs["g"], inputs["W1t"], inputs["b1t"],
        inputs["W1p"], inputs["b1p"], inputs["W2"], inputs["b2"],
    )
    res = run_bass_kernel_spmd(
        _get_nc(), in_maps, core_ids=list(range(N_CORES)), trace=trace
    )
    return _gather(res.results), res


def kernel(**inputs) -> np.ndarray:
    out, _ = _run(inputs, trace=False)
    return out


# revision 44
# speedup vs baseline: 1.0158x; 1.0158x over previous
"""RNN-T joint network (dense MLP) Trainium2 Bass kernel.

Math (per batch row n):
    h = relu(x @ W1.T + b1)     x = concat(f, g)   # [N, 512]
    y = h @ W2.T + b2                              # [N, 29]

Strategy: data-parallel over batch N=32768 across 8 NeuronCores (4096
rows/core); weights replicated.

Both layers run on the PE in fp8e4 (e4m3) DoubleRow mode (0.5
cycles/row, 2 k-rows per slot-pair) with 3-term error compensation:

    x ~= x8 + xlo          (both e4m3, exact split of the f32 value)
    W ~= (Whi + Wlo)/S     (e4m3 at device scale S; Wlo = residual)
    x @ W = (x8 @ (Whi + Wlo) + xlo @ Whi) / S     [+ O(eps^2) dropped]

Layer 1: every DoubleRow matmul carries two k-tiles (one per slot), so
per j-tile and 512-row chunk layer 1 is 12 DR instructions covering
K=1344 (10 full 128-tiles + one 64-row tile packed by stacking
[Whi;Wlo] vertically in one slot), at 256 PE cycles each — 3072 cycles
vs 5632 for the fp32r baseline: a-terms (x8 @ Whi, 5 DR), the b-term
(x8 @ Wlo) only for the g-part k-tile pair 8-9 whose weights have 3.2x
the variance of the f-part (dropping the f-part Wlo raises the measured
rel err to a still-safe 1.52e-2), the stacked k-tile-10 DR, and c-terms
(xlo @ Whi, 5 DR) which reuse the a-term weight tiles in SBUF.

Layer 2 (chunks 0..6): h is split on-device into h8 (2nd ScalarE
activation pass, fp8 out) + hlo (VectorE subtract, fp8 out) and the
29-wide projection runs as 6 DR matmuls (1536 cycles vs 2048 fp32r).
The last chunk keeps the fp32r path so the drain doesn't wait on the
h8/hlo chain.  Labels padded 29->32 (DR weight AP step must be %16).

Timeline tuning (cost-model driven): the PE p-state ramp resets on any
engine gap (3us of half-clock matmuls each time), so scratch "warmup"
DR matmuls bridge the DMA fill and the final-chunk activation latency;
the fill DMAs ride one queue (SP) in just-in-time order.
"""

import numpy as np
import ml_dtypes

import concourse.bacc as bacc
import concourse.bass as bass  # noqa: F401
import concourse.mybir as mybir
from concourse import tile
from concourse.bass_utils import run_bass_kernel_spmd

TRANS_H, PRED_H, JOINT_H, NUM_LABELS = 1024, 320, 512, 29
LAB_PAD = 32
BATCH = 32768
N_CORES = 8
N_PER_CORE = BATCH // N_CORES          # 4096
K_TOTAL = TRANS_H + PRED_H             # 1344 = 10*128 + 64
K_FULL = 10                            # full 128-row k-tiles
K_REM = K_TOTAL - K_FULL * 128         # 64
J_TILES = JOINT_H // 128               # 4
N_CHUNK = 512                          # PSUM-bank / fp32r moving limit
N_CHUNKS = N_PER_CORE // N_CHUNK       # 8
X_SLOTS = 22                           # 10 x8 + x8dup + xlohalf + 10 xlo
N_DR = 7                               # weight DR slots per j (a:5, b:1, t10:1)
W_SCALE = 4096.0
W2_SCALE = 2048.0
N_WARM = 14                            # fill-bridging PE warmup matmuls
N_WARM_TAIL = 4                        # drain-bridging warmups (chunk 7 l2)

F32 = mybir.dt.float32
F32R = mybir.dt.float32r
F8 = mybir.dt.float8e4
DR = mybir.MatmulPerfMode.DoubleRow
E4NP = ml_dtypes.float8_e4m3

_NC_CACHE = {}


def _build_bass():
    """Single-core Bass program (same NEFF runs SPMD on 8 cores)."""
    nc = bacc.Bacc(None)

    xq = nc.dram_tensor("xq", [X_SLOTS * 128, N_PER_CORE], F8, kind="ExternalInput")
    w1 = nc.dram_tensor("w1", [128, J_TILES, N_DR, 2, 128], F8, kind="ExternalInput")
    b1 = nc.dram_tensor("b1", [JOINT_H, 1], F32, kind="ExternalInput")
    w2T = nc.dram_tensor("w2T", [JOINT_H, NUM_LABELS], F32R, kind="ExternalInput")
    w2dr = nc.dram_tensor("w2dr", [128, 4, 2, LAB_PAD], F8, kind="ExternalInput")
    b2 = nc.dram_tensor("b2", [NUM_LABELS, 1], F32, kind="ExternalInput")
    yT = nc.dram_tensor("yT", [NUM_LABELS, N_PER_CORE], F32, kind="ExternalOutput")

    xq3 = xq.rearrange("(s p) n -> p s n", p=128)     # [128, 22, N]

    with tile.TileContext(nc) as tc:
        with (
            tc.tile_pool(name="consts", bufs=1) as consts,
            tc.tile_pool(name="xpool", bufs=3) as xpool,
            tc.tile_pool(name="lopool", bufs=3) as lopool,
            tc.tile_pool(name="hpool", bufs=2) as hpool,
            tc.tile_pool(name="h8pool", bufs=2) as h8pool,
            tc.tile_pool(name="opool", bufs=2) as opool,
            tc.tile_pool(name="psum_h", bufs=5, space="PSUM") as psum_h,
            tc.tile_pool(name="psum_y", bufs=2, space="PSUM") as psum_y,
            tc.tile_pool(name="psum_w", bufs=1, space="PSUM") as psum_w,
        ):
            # ---- constants: single SP queue, just-in-time order ----
            w1_sb = consts.tile([128, J_TILES, N_DR, 2, 128], F8, name="w1_sb", tag="w1")
            b1_sb = consts.tile([128, J_TILES], F32, name="b1_sb", tag="b1")
            w2_sb = consts.tile([128, J_TILES, NUM_LABELS], F32R, name="w2_sb", tag="w2")
            w2dr_sb = consts.tile([128, 4, 2, LAB_PAD], F8, name="w2dr_sb", tag="w2dr")
            b2_sb = consts.tile([NUM_LABELS, 1], F32, name="b2_sb", tag="b2")

            x8_c0 = xpool.tile([128, 12, N_CHUNK], F8, name="x8_sb", tag="x8")
            xlo_c0 = lopool.tile([128, 10, N_CHUNK], F8, name="xlo_sb", tag="xlo")

            nc.sync.dma_start(out=w1_sb[:, 0], in_=w1[:, 0])
            nc.sync.dma_start(out=x8_c0[:, 0:10], in_=xq3[:, 0:10, 0:N_CHUNK])
            nc.sync.dma_start(out=x8_c0[:, 10:12], in_=xq3[:, 10:12, 0:N_CHUNK])
            nc.sync.dma_start(out=xlo_c0[:, 0:6], in_=xq3[:, 12:18, 0:N_CHUNK])
            nc.sync.dma_start(out=b1_sb, in_=b1.rearrange("(j p) o -> p (j o)", p=128))
            nc.sync.dma_start(out=w1_sb[:, 1], in_=w1[:, 1])
            nc.sync.dma_start(out=xlo_c0[:, 6:10], in_=xq3[:, 18:22, 0:N_CHUNK])
            nc.sync.dma_start(out=w1_sb[:, 2], in_=w1[:, 2])
            nc.sync.dma_start(out=w1_sb[:, 3], in_=w1[:, 3])
            xlo_c1 = lopool.tile([128, 10, N_CHUNK], F8, name="xlo_sb", tag="xlo")
            nc.sync.dma_start(out=xlo_c1[:, 0:6], in_=xq3[:, 12:18, N_CHUNK:2 * N_CHUNK])
            nc.sync.dma_start(out=xlo_c1[:, 6:10], in_=xq3[:, 18:22, N_CHUNK:2 * N_CHUNK])
            x8_c1 = xpool.tile([128, 12, N_CHUNK], F8, name="x8_sb", tag="x8")
            nc.sync.dma_start(out=x8_c1, in_=xq3[:, 0:12, N_CHUNK:2 * N_CHUNK])
            nc.sync.dma_start(out=w2dr_sb, in_=w2dr[:, :, :, :])
            nc.sync.dma_start(out=b2_sb, in_=b2[:, :])
            nc.sync.dma_start(
                out=w2_sb, in_=w2T.rearrange("(j p) l -> p j l", p=128)
            )

            # ---- PE warmup: scratch DR matmuls keep the p-state ramp hot
            # across the DMA fill (any PE gap resets 3us of full clock).
            warm_sb = consts.tile([128, 2, N_CHUNK], F8, name="warm_sb", tag="warm")
            nc.vector.memset(warm_sb, 0.0)
            pw = psum_w.tile([128, N_CHUNK], F32, name="pw", tag="pw")

            def emit_warmups(n):
                for _ in range(n):
                    nc.tensor.matmul(
                        pw, lhsT=warm_sb[:, :, 0:128], rhs=warm_sb,
                        start=True, stop=True, perf_mode=DR,
                    )

            emit_warmups(N_WARM)

            # ---- main loop over batch chunks of 512 rows ----
            # Layer 2 of chunk c-1 is emitted mid-chunk c so the PE never
            # idles on the j3 activation / h8 / hlo chain.
            pending = None  # (h_tiles, h8_pairs, hlo_pairs, n0) of prev chunk
            for c in range(N_CHUNKS):
                n0 = c * N_CHUNK
                if c > 1:
                    x8_sb = xpool.tile([128, 12, N_CHUNK], F8, name="x8_sb", tag="x8")
                    nc.sync.dma_start(out=x8_sb, in_=xq3[:, 0:12, n0:n0 + N_CHUNK])
                    xlo_sb = lopool.tile([128, 10, N_CHUNK], F8, name="xlo_sb", tag="xlo")
                    nc.sync.dma_start(out=xlo_sb[:, 0:6], in_=xq3[:, 12:18, n0:n0 + N_CHUNK])
                    nc.sync.dma_start(out=xlo_sb[:, 6:10], in_=xq3[:, 18:22, n0:n0 + N_CHUNK])
                elif c == 1:
                    x8_sb, xlo_sb = x8_c1, xlo_c1
                else:
                    x8_sb, xlo_sb = x8_c0, xlo_c0

                last = c == N_CHUNKS - 1
                h_tiles, h8_pairs, hlo_pairs = [], [], []
                if not last:
                    for jp in range(2):
                        h8_pairs.append(h8pool.tile(
                            [128, 2, N_CHUNK], F8, name=f"h8_{jp}", tag=f"h8_{jp}"))
                        hlo_pairs.append(h8pool.tile(
                            [128, 2, N_CHUNK], F8, name=f"hlo_{jp}", tag=f"hlo_{jp}"))
                def emit_block(j, ph_out, c0, c1):
                    # a-terms: x8 @ Whi, 5 DR pairs over k-tiles 0..9
                    for q in range(5):
                        nc.tensor.matmul(
                            ph_out, lhsT=w1_sb[:, j, q],
                            rhs=x8_sb[:, 2 * q:2 * q + 2, c0:c1],
                            start=(q == 0), stop=False, perf_mode=DR,
                        )
                    # b-term: x8 @ Wlo for the g-part k-tile pair (8,9)
                    # only — f-part weights have 3.2x less variance and their
                    # Wlo is dropped (measured rel err 1.52e-2 vs 2e-2 gate)
                    nc.tensor.matmul(
                        ph_out, lhsT=w1_sb[:, j, 5], rhs=x8_sb[:, 8:10, c0:c1],
                        start=False, stop=False, perf_mode=DR,
                    )
                    # k-tile 10 (64 rows): slot0 [Whi10;Wlo10]@[x8;x8],
                    # slot1 [Whi10;0]@[xlo;0]
                    nc.tensor.matmul(
                        ph_out, lhsT=w1_sb[:, j, 6], rhs=x8_sb[:, 10:12, c0:c1],
                        start=False, stop=False, perf_mode=DR,
                    )
                    # c-terms: xlo @ Whi (reuses a-term weight tiles)
                    for q in range(5):
                        nc.tensor.matmul(
                            ph_out, lhsT=w1_sb[:, j, q],
                            rhs=xlo_sb[:, 2 * q:2 * q + 2, c0:c1],
                            start=False, stop=(q == 4), perf_mode=DR,
                        )

                c1_phs = None
                if c == 1:
                    # c-first: run xlo-only terms while x8_c1 is in flight
                    c1_phs = [psum_h.tile([128, N_CHUNK], F32,
                                          name=f"ph_{j}", tag="ph")
                              for j in range(J_TILES)]
                    for j in range(J_TILES):
                        for q in range(5):
                            nc.tensor.matmul(
                                c1_phs[j], lhsT=w1_sb[:, j, q],
                                rhs=xlo_sb[:, 2 * q:2 * q + 2, :],
                                start=(q == 0), stop=False, perf_mode=DR,
                            )
                for j in range(J_TILES):
                    if c == 1:
                        ph = c1_phs[j]
                        for q in range(5):
                            nc.tensor.matmul(
                                ph, lhsT=w1_sb[:, j, q],
                                rhs=x8_sb[:, 2 * q:2 * q + 2, :],
                                start=False, stop=False, perf_mode=DR,
                            )
                        nc.tensor.matmul(
                            ph, lhsT=w1_sb[:, j, 5], rhs=x8_sb[:, 8:10, :],
                            start=False, stop=False, perf_mode=DR,
                        )
                        nc.tensor.matmul(
                            ph, lhsT=w1_sb[:, j, 6], rhs=x8_sb[:, 10:12, :],
                            start=False, stop=True, perf_mode=DR,
                        )
                        h_sb = hpool.tile([128, N_CHUNK], F32R,
                                          name=f"h_{j}", tag=f"h_{j}")
                        nc.scalar.activation(
                            h_sb, ph, mybir.ActivationFunctionType.Relu,
                            bias=b1_sb[:, j:j + 1], scale=1.0 / W_SCALE,
                        )
                        h_tiles.append(h_sb)
                        h8 = h8_pairs[j // 2][:, j % 2, :]
                        nc.scalar.activation(
                            h8, ph, mybir.ActivationFunctionType.Relu,
                            bias=b1_sb[:, j:j + 1], scale=1.0 / W_SCALE,
                        )
                        nc.vector.tensor_tensor(
                            out=hlo_pairs[j // 2][:, j % 2, :], in0=h_sb, in1=h8,
                            op=mybir.AluOpType.subtract,
                        )
                        if j == 2 and pending is not None:
                            pending_py = _emit_l2_matmuls(nc, psum_y, w2dr_sb, *pending)
                        if j == 3 and pending is not None:
                            _emit_l2_out(nc, opool, b2_sb, yT, pending_py, pending[-1])
                            pending = None
                        continue
                    if last and j == 3:
                        # half-width groups on two banks: the 385ns half
                        # activations overlap the DR stream instead of
                        # gating the whole drain at 612ns
                        h_sb = hpool.tile([128, N_CHUNK], F32R,
                                          name="h_3", tag="h_3")
                        pha = psum_h.tile([128, N_CHUNK], F32, name="ph_3a", tag="ph")
                        emit_block(3, pha[:, 0:256], 0, 256)
                        nc.scalar.activation(
                            h_sb[:, 0:256], pha[:, 0:256],
                            mybir.ActivationFunctionType.Relu,
                            bias=b1_sb[:, 3:4], scale=1.0 / W_SCALE,
                        )
                        phb = psum_h.tile([128, N_CHUNK], F32, name="ph_3b", tag="ph")
                        emit_block(3, phb[:, 0:256], 256, 512)
                        nc.scalar.activation(
                            h_sb[:, 256:512], phb[:, 0:256],
                            mybir.ActivationFunctionType.Relu,
                            bias=b1_sb[:, 3:4], scale=1.0 / W_SCALE,
                        )
                        h_tiles.append(h_sb)
                        if pending is not None:
                            # c6's y bias+scale on DVE so it never contends
                            # with the final ACT chain
                            py6, n6 = pending_py, pending[-1]
                            y6 = opool.tile([NUM_LABELS, N_CHUNK], F32,
                                            name="y_sb", tag="y")
                            nc.vector.tensor_scalar(
                                out=y6, in0=py6[0:NUM_LABELS, :],
                                scalar1=1.0 / W2_SCALE, scalar2=b2_sb,
                                op0=mybir.AluOpType.mult,
                                op1=mybir.AluOpType.add,
                            )
                            nc.sync.dma_start(
                                out=yT[:, n6:n6 + N_CHUNK], in_=y6)
                            pending = None
                        continue
                    ph = psum_h.tile([128, N_CHUNK], F32, name=f"ph_{j}", tag="ph")
                    emit_block(j, ph, 0, N_CHUNK)
                    h_sb = hpool.tile([128, N_CHUNK], F32R, name=f"h_{j}", tag=f"h_{j}")
                    nc.scalar.activation(
                        h_sb, ph, mybir.ActivationFunctionType.Relu,
                        bias=b1_sb[:, j:j + 1], scale=1.0 / W_SCALE,
                    )
                    h_tiles.append(h_sb)
                    if not last:
                        h8 = h8_pairs[j // 2][:, j % 2, :]
                        nc.scalar.activation(
                            h8, ph, mybir.ActivationFunctionType.Relu,
                            bias=b1_sb[:, j:j + 1], scale=1.0 / W_SCALE,
                        )
                        nc.vector.tensor_tensor(
                            out=hlo_pairs[j // 2][:, j % 2, :], in0=h_sb, in1=h8,
                            op=mybir.AluOpType.subtract,
                        )
                    if j == 2 and pending is not None:
                        pending_py = _emit_l2_matmuls(nc, psum_y, w2dr_sb, *pending)
                    if j == 3 and pending is not None:
                        # y-act after act1-j3 so it never delays the h chain
                        _emit_l2_out(nc, opool, b2_sb, yT, pending_py, pending[-1])
                        pending = None

                pending = (h_tiles, h8_pairs, hlo_pairs, n0)

            # drain: bridge the final activation latency, then fp32r layer 2
            # for the last chunk (no h8/hlo dependency in the tail).
            emit_warmups(N_WARM_TAIL)
            h_tiles, _, _, n0 = pending
            y_sb = opool.tile([NUM_LABELS, N_CHUNK], F32, name="y_sb", tag="y")
            py_a = psum_y.tile([NUM_LABELS, 256], F32, name="py_a", tag="py")
            for j in range(J_TILES):
                nc.tensor.matmul(
                    py_a, lhsT=w2_sb[:, j, :], rhs=h_tiles[j][:, 0:256],
                    start=(j == 0), stop=(j == J_TILES - 1),
                )
            nc.scalar.activation(
                y_sb[:, 0:256], py_a, mybir.ActivationFunctionType.Identity,
                bias=b2_sb,
            )
            py_b = psum_y.tile([NUM_LABELS, 256], F32, name="py_b", tag="py")
            for j in range(J_TILES):
                nc.tensor.matmul(
                    py_b, lhsT=w2_sb[:, j, :], rhs=h_tiles[j][:, 256:512],
                    start=(j == 0), stop=(j == J_TILES - 1),
                )
            nc.vector.tensor_scalar_add(
                out=y_sb[:, 256:512], in0=py_b, scalar1=b2_sb
            )
            # final y rides SP (dge 650 vs 784 on ACT; SP is idle at drain)
            nc.sync.dma_start(out=yT[:, n0:n0 + N_CHUNK], in_=y_sb)

    nc.finalize()
    return nc


def _emit_l2_matmuls(nc, psum_y, w2dr_sb, h_tiles, h8_pairs, hlo_pairs, n0):
    """fp8 DR layer 2 matmuls: py = h8 @ (W2hi+W2lo) + hlo @ W2hi."""
    py = psum_y.tile([LAB_PAD, N_CHUNK], F32, name="py", tag="py")
    nc.tensor.matmul(py, lhsT=w2dr_sb[:, 0], rhs=h8_pairs[0],
                     start=True, stop=False, perf_mode=DR)
    nc.tensor.matmul(py, lhsT=w2dr_sb[:, 1], rhs=h8_pairs[1],
                     start=False, stop=False, perf_mode=DR)
    nc.tensor.matmul(py, lhsT=w2dr_sb[:, 2], rhs=h8_pairs[0],
                     start=False, stop=False, perf_mode=DR)
    nc.tensor.matmul(py, lhsT=w2dr_sb[:, 3], rhs=h8_pairs[1],
                     start=False, stop=False, perf_mode=DR)
    nc.tensor.matmul(py, lhsT=w2dr_sb[:, 0], rhs=hlo_pairs[0],
                     start=False, stop=False, perf_mode=DR)
    nc.tensor.matmul(py, lhsT=w2dr_sb[:, 1], rhs=hlo_pairs[1],
                     start=False, stop=True, perf_mode=DR)
    return py


def _emit_l2_out(nc, opool, b2_sb, yT, py, n0):
    y_sb = opool.tile([NUM_LABELS, N_CHUNK], F32, name="y_sb", tag="y")
    nc.scalar.activation(
        y_sb, py[0:NUM_LABELS, :], mybir.ActivationFunctionType.Identity,
        bias=b2_sb, scale=1.0 / W2_SCALE,
    )
    nc.scalar.dma_start(out=yT[:, n0:n0 + N_CHUNK], in_=y_sb)


def _get_nc():
    if "nc" not in _NC_CACHE:
        _NC_CACHE["nc"] = _build_bass()
    return _NC_CACHE["nc"]


def _q8(a):
    return np.asarray(a, dtype=E4NP)


def _prep_in_maps(f, g, W1t, b1t, W1p, b1p, W2, b2):
    f2 = np.asarray(f, np.float32).reshape(BATCH, TRANS_H)
    g2 = np.asarray(g, np.float32).reshape(BATCH, PRED_H)
    x = np.concatenate([f2, g2], axis=1)            # [BATCH, 1344]

    x8 = _q8(x)                                     # e4m3, device scale 1
    xlo = _q8(x - x8.astype(np.float32))            # e4m3 residual, scale 1

    W1 = np.concatenate(
        [np.asarray(W1t, np.float32), np.asarray(W1p, np.float32)], axis=1
    ).T                                             # [1344, 512]
    Whi = _q8(W1 * W_SCALE)                         # device scale 4096
    Wlo = _q8(W1 * W_SCALE - Whi.astype(np.float32))

    # layer-1 weight DR-pair tensor [p, j, dr, slot, col]
    w1dr = np.zeros((128, J_TILES, N_DR, 2, 128), dtype=E4NP)
    Whi_p = np.zeros((11 * 128, JOINT_H), dtype=E4NP)
    Wlo_p = np.zeros((11 * 128, JOINT_H), dtype=E4NP)
    Whi_p[:K_TOTAL] = Whi
    Wlo_p[:K_TOTAL] = Wlo
    for j in range(J_TILES):
        cols = slice(j * 128, (j + 1) * 128)
        for q in range(5):
            w1dr[:, j, q, 0] = Whi_p[(2 * q) * 128:(2 * q + 1) * 128, cols]
            w1dr[:, j, q, 1] = Whi_p[(2 * q + 1) * 128:(2 * q + 2) * 128, cols]
        w1dr[:, j, 5, 0] = Wlo_p[8 * 128:9 * 128, cols]
        w1dr[:, j, 5, 1] = Wlo_p[9 * 128:10 * 128, cols]
        # k-tile 10: slot0 = [Whi10; Wlo10] stacked, slot1 = [Whi10; 0]
        w1dr[:K_REM, j, 6, 0] = Whi[K_FULL * 128:, cols]
        w1dr[K_REM:2 * K_REM, j, 6, 0] = Wlo[K_FULL * 128:, cols]
        w1dr[:K_REM, j, 6, 1] = Whi[K_FULL * 128:, cols]

    # layer-2 weight DR tensor [p, {hi01,hi23,lo01,lo23}, slot, lab]
    W2T = np.asarray(W2, np.float32).T              # [512, 29]
    W2hi = _q8(W2T * W2_SCALE)
    W2lo = _q8(W2T * W2_SCALE - W2hi.astype(np.float32))
    w2drh = np.zeros((128, 4, 2, LAB_PAD), dtype=E4NP)
    for jp in range(2):
        for s in range(2):
            rows = slice((2 * jp + s) * 128, (2 * jp + s + 1) * 128)
            w2drh[:, jp, s, :NUM_LABELS] = W2hi[rows]
            w2drh[:, 2 + jp, s, :NUM_LABELS] = W2lo[rows]

    b1c = (np.asarray(b1t, np.float32) + np.asarray(b1p, np.float32)).reshape(
        JOINT_H, 1
    )
    w2c = np.ascontiguousarray(W2T)
    b2c = np.asarray(b2, np.float32).reshape(NUM_LABELS, 1)

    in_maps = []
    for core in range(N_CORES):
        sl = slice(core * N_PER_CORE, (core + 1) * N_PER_CORE)
        x8c = x8[sl]                                # [4096, 1344]
        xloc = xlo[sl]
        xqc = np.zeros((X_SLOTS, 128, N_PER_CORE), dtype=E4NP)
        for t in range(K_FULL):
            xqc[t] = x8c[:, t * 128:(t + 1) * 128].T
        # slot 10: x8 tile-10 duplicated vertically; slot 11: xlo tile-10 + 0s
        xqc[10, :K_REM] = x8c[:, K_FULL * 128:].T
        xqc[10, K_REM:] = x8c[:, K_FULL * 128:].T
        xqc[11, :K_REM] = xloc[:, K_FULL * 128:].T
        for t in range(K_FULL):
            xqc[12 + t] = xloc[:, t * 128:(t + 1) * 128].T
        in_maps.append({
            "xq": xqc.reshape(X_SLOTS * 128, N_PER_CORE),
            "w1": w1dr, "b1": b1c, "w2T": w2c, "w2dr": w2drh, "b2": b2c,
        })
    return in_maps


def _gather(results):
    y = np.empty((1, BATCH, NUM_LABELS), np.float32)
    for core, r in enumerate(results):
        y[0, core * N_PER_CORE:(core + 1) * N_PER_CORE] = r["yT"].T
    return y


def _run(inputs, trace=False):
    in_maps = _prep_in_maps(
        inputs["f"], inputs["g"], inputs["W1t"], inputs["b1t"],
        inputs["W1p"], inputs["b1p"], inputs["W2"], inputs["b2"],
    )
    res = run_bass_kernel_spmd(
        _get_nc(), in_maps, core_ids=list(range(N_CORES)), trace=trace
    )
    return _gather(res.results), res


def kernel(**inputs) -> np.ndarray:
    out, _ = _run(inputs, trace=False)
    return out
